# revision 11
# baseline (speedup 1.0000x reference)
"""Density-aware Chamfer distance kernel for Trainium2 (Bass/Tile).

Contract: kernel(xyz1, xyz2) takes FULL inputs (8, 4096, 3) fp32 and
returns the FULL scalar output, sharding batch-parallel across 8
NeuronCores (1 point-cloud pair per core).

Math note (avoids argmin indices / gathers entirely):
  loss_b = 1 - (S1 + S2) / (2N)  with
  S_d = sum_j T[j] * mask[j] / (c[j] + eps)
  c[j]  = #rows whose argmin is column j       (count)
  T[j]  = sum of exp(-1000*dmin_i) over rows i with argmin j
Both c and T are column sums of the one-hot argmin indicator
S[i,j] = [D[i,j] == rowmin_i], computed as 1 - Z with
Z = Sign(D - rowmin) in {0,1} and accumulated on the tensor engine
via Z^T @ [ones, exp] (complement form).
"""

import numpy as np

B = 8
N = 4096
NCORES = 8
ALPHA = 1000.0
EPS = 1e-6

K = 6                # augmented contraction dim (xyz, norms, ones, tilt)
TILT = 2.0 ** -37    # tie-breaking tilt: D[i,j] += j*TILT (first-min wins)
P = 128              # rows per strip
NSTRIP = N // P      # 32 strips per direction
GROUP = 1024         # D columns per PSUM group tile (2 banks)
NGROUP = N // GROUP  # 4
CHUNK = 512          # max fp32 matmul moving free dim
SUB = 128            # czT subchunk (matmul M limit)

_cache = {}
last_run_info = {}


def _build_nc():
    import concourse.bacc as bacc
    import concourse.tile as tile
    from concourse import mybir

    f32 = mybir.dt.float32
    bf16 = mybir.dt.bfloat16
    X = mybir.AxisListType.X
    Alu = mybir.AluOpType
    Act = mybir.ActivationFunctionType

    nc = bacc.Bacc("TRN2", target_bir_lowering=False, debug=False)

    u_dram = [None, None]
    v_dram = [None, None]
    for d in range(2):
        u_dram[d] = nc.declare_dram_parameter(f"u{d}", [K, N], f32, isOutput=False)
        v_dram[d] = nc.declare_dram_parameter(f"v{d}", [K, N], f32, isOutput=False)
    out_dram = nc.declare_dram_parameter("out", [1, 1], f32, isOutput=True)

    with tile.TileContext(nc) as tc:
        with (
            tc.tile_pool(name="uv", bufs=1) as uv_pool,
            tc.tile_pool(name="persist", bufs=1) as persist,
            tc.tile_pool(name="zbuf", bufs=2) as zpool,
            tc.tile_pool(name="small", bufs=4) as small,
            tc.tile_pool(name="ep", bufs=1) as ep,
            tc.tile_pool(name="ps", bufs=4, space="PSUM") as psum,
        ):
            # load U/V operands (K=5 on partitions)
            u_sb = [None, None]
            v_sb = [None, None]
            for d in range(2):
                u_sb[d] = uv_pool.tile([K, N], f32, name=f"u{d}sb", tag=f"u{d}")
                v_sb[d] = uv_pool.tile([K, N], f32, name=f"v{d}sb", tag=f"v{d}")
                nc.sync.dma_start(out=u_sb[d][:], in_=u_dram[d][:])
                nc.sync.dma_start(out=v_sb[d][:], in_=v_dram[d][:])

            # persistent per-direction accumulation slabs
            # per strip: 64 cols of [cnt-complement, mass-complement] per
            # j-subchunk + 2 cols [128, se_t] from the all-ones lhsT matmul
            ctw = 2 * (N // SUB) + 2
            cz_slab = [persist.tile([P, NSTRIP, ctw], f32,
                                    name=f"czslab{d}", tag=f"cz{d}")
                       for d in range(2)]  # [P, 32, 66]
            ones_sb = persist.tile([P, SUB], bf16, name="ones_sb")
            nc.vector.memset(ones_sb[:], 1.0)
            spart = [None, None]

            for d in range(2):
                U, V = u_sb[d], v_sb[d]
                for t in range(NSTRIP):
                    lhsT = U[:, t * P:(t + 1) * P]
                    pm = small.tile([P, NGROUP], f32, name="pm", tag="pm")
                    zt = zpool.tile([P, N], bf16, name="zt", tag="z")
                    dgs = []
                    for g in range(NGROUP):
                        dg = psum.tile([P, GROUP], f32, name="dg", tag="dg")
                        dgs.append(dg)
                        for c in range(GROUP // CHUNK):
                            j0 = g * GROUP + c * CHUNK
                            nc.tensor.matmul(
                                dg[:, c * CHUNK:(c + 1) * CHUNK],
                                lhsT=lhsT,
                                rhs=V[:, j0:j0 + CHUNK],
                                start=True, stop=True,
                            )
                        nc.vector.tensor_reduce(
                            pm[:, g:g + 1], dg[:], axis=X, op=Alu.min)
                    rowmin = small.tile([P, 1], f32, name="rowmin", tag="rm")
                    nc.vector.tensor_reduce(rowmin[:], pm[:], axis=X, op=Alu.min)
                    negmin = small.tile([P, 1], f32, name="negmin", tag="nm")
                    nc.vector.tensor_scalar_mul(negmin[:], rowmin[:], -1.0)
                    wt = small.tile([P, 2], bf16, name="wt", tag="w")
                    nc.vector.memset(wt[:, 0:1], 1.0)
                    nc.scalar.activation(
                        wt[:, 1:2], rowmin[:], Act.Exp, scale=-ALPHA)
                    for g in range(NGROUP):
                        nc.scalar.activation(
                            zt[:, g * GROUP:(g + 1) * GROUP], dgs[g][:],
                            Act.Sign, bias=negmin[:], scale=1.0)
                    ct = psum.tile([P, ctw], f32, name="ct", tag="dg")
                    for s in range(N // SUB):
                        nc.tensor.matmul(
                            ct[:, 2 * s:2 * s + 2],
                            lhsT=zt[:, s * SUB:(s + 1) * SUB],
                            rhs=wt[:],
                            start=True, stop=True,
                        )
                    # se_t with the same systolic accumulation tree as
                    # cz1_t, replicated to all partitions by the ones lhsT
                    nc.tensor.matmul(
                        ct[:, 2 * (N // SUB):ctw],
                        lhsT=ones_sb[:],
                        rhs=wt[:],
                        start=True, stop=True,
                    )
                    nc.vector.tensor_copy(cz_slab[d][:, t, :], ct[:])

                # ---- per-direction epilogue ----
                nsub = N // SUB
                # counts: c[j] = N - sum_t cz0_t[j]  (exact integer sums)
                cz0 = cz_slab[d][:, :, 0:2 * nsub].rearrange(
                    "p t (s two) -> p s two t", two=2)[:, :, 0, :]  # [P,s,t]
                cz0sum = ep.tile([P, nsub], f32)
                nc.vector.tensor_reduce(cz0sum[:], cz0, axis=X, op=Alu.add)
                # per-strip row-sums of exp (PE-computed, same tree as
                # cz1, already replicated across partitions)
                se_row = cz_slab[d][:, :, ctw - 1]
                # T[j] = sum_t (se_t - cz1_t[j]): small differences per strip
                tneg = ep.tile([P, nsub, NSTRIP], f32)
                for s in range(nsub):
                    nc.vector.scalar_tensor_tensor(
                        out=tneg[:, s, :],
                        in0=cz_slab[d][:, :, 2 * s + 1],
                        scalar=-1.0, in1=se_row,
                        op0=Alu.mult, op1=Alu.add)
                tj = ep.tile([P, nsub], f32)
                nc.vector.tensor_reduce(tj[:], tneg[:], axis=X, op=Alu.add)
                c1 = ep.tile([P, nsub], f32)
                nc.vector.tensor_scalar(
                    c1[:], cz0sum[:], -1.0, float(N), op0=Alu.mult, op1=Alu.add)
                c1e = ep.tile([P, nsub], f32)
                nc.vector.tensor_scalar_add(c1e[:], c1[:], EPS)
                r = ep.tile([P, nsub], f32)
                nc.vector.reciprocal(r[:], c1e[:])
                mask = ep.tile([P, nsub], f32)
                nc.vector.tensor_scalar_min(mask[:], c1[:], 1.0)
                rm = ep.tile([P, nsub], f32)
                nc.vector.tensor_mul(rm[:], r[:], mask[:])
                junk = ep.tile([P, nsub], f32)
                sp = ep.tile([P, 1], f32, name=f"sp{d}", tag=f"sp{d}")
                spart[d] = sp
                nc.vector.tensor_mul(junk[:], tj[:], rm[:])
                nc.vector.tensor_reduce(sp[:], junk[:], axis=X, op=Alu.add)

            sall = ep.tile([P, 1], f32)
            nc.vector.tensor_add(sall[:], spart[0][:], spart[1][:])
            stot = ep.tile([P, 1], f32)
            nc.gpsimd.partition_all_reduce(
                stot[:], sall[:], channels=P, reduce_op=_reduce_op_add())
            nc.sync.dma_start(out=out_dram[:], in_=stot[0:1, 0:1])

    nc.compile()
    return nc


def _reduce_op_add():
    from concourse import bass_isa
    return bass_isa.ReduceOp.add


def _make_uv(x, y):
    """U=( -2x | |x|^2 | 1 | 1 ), V=( y | 1 | |y|^2 | j*TILT ), [6, N] fp32.

    D[i,j] = U[:,i] . V[:,j] = |x_i|^2 + |y_j|^2 - 2 x_i.y_j + j*TILT
    The tilt breaks exact fp32 ties so the min lands on the first
    (smallest-j) minimum, matching jnp.argmin tie semantics.
    """
    n = x.shape[0]
    u = np.empty((K, n), np.float32)
    v = np.empty((K, n), np.float32)
    u[0:3] = (-2.0 * x.T).astype(np.float32)
    u[3] = np.sum(x * x, axis=1, dtype=np.float32)
    u[4] = 1.0
    u[5] = 1.0
    v[0:3] = y.T.astype(np.float32)
    v[3] = 1.0
    v[4] = np.sum(y * y, axis=1, dtype=np.float32)
    v[5] = (np.arange(n) * TILT).astype(np.float32)
    return u, v


def kernel(xyz1: np.ndarray, xyz2: np.ndarray) -> np.ndarray:
    from concourse.bass_utils import run_bass_kernel_spmd

    xyz1 = np.asarray(xyz1, np.float32)
    xyz2 = np.asarray(xyz2, np.float32)
    assert xyz1.shape == (B, N, 3) and xyz2.shape == (B, N, 3)

    if "nc" not in _cache:
        _cache["nc"] = _build_nc()
    nc = _cache["nc"]

    in_maps = []
    for b in range(B):
        u0, v0 = _make_uv(xyz1[b], xyz2[b])   # direction 1: rows=x1, cols=x2
        u1, v1 = _make_uv(xyz2[b], xyz1[b])   # direction 2: rows=x2, cols=x1
        in_maps.append({"u0": u0, "v0": v0, "u1": u1, "v1": v1})

    trace = bool(last_run_info.get("want_trace"))
    res = run_bass_kernel_spmd(
        nc, in_maps, core_ids=list(range(NCORES)), trace=trace)
    last_run_info["exec_time_ns"] = res.exec_time_ns
    last_run_info["profile_json"] = res.profile_json

    s = np.array([res.results[b]["out"][0, 0] for b in range(B)], np.float64)
    loss = 1.0 - s.sum() / (B * 2 * N)
    return np.float32(loss)
